# revision 1
# baseline (speedup 1.0000x reference)
"""Trainium2 Bass kernel for the CondConv-style dense CNN.

Model (per sample b):
  att[b]  = softmax(MLP(avgpool(scene_knowledge[b])) / 30)        # [16]
  agg_w   = sum_k att[b,k] * weight[k]                            # [256,256,3,3]
  out[b]  = conv3x3_same(x[b], agg_w) + att[b] @ bias + x[b]

Sharding: 8 cores = 4 sample-pairs (g) x 2 output-channel halves (h).
Each core processes 2 samples and 128 output channels.  All math runs
on-device; the host only does layout transforms + shard slicing.
"""

import sys
import numpy as np

sys.path.insert(0, "/opt/trn_rl_repo")

import concourse.bass as bass
import concourse.mybir as mybir
from concourse.tile import TileContext
from concourse.masks import make_identity

F32 = mybir.dt.float32
BF16 = mybir.dt.bfloat16
AX = mybir.AxisListType
OP = mybir.AluOpType
ACT = mybir.ActivationFunctionType

TEMPERATURE = 30.0
NCORES = 8


def build_program() -> bass.Bass:
    nc = bass.Bass()

    x2 = nc.declare_dram_parameter("x2", [2, 256, 64, 64], F32, isOutput=False)
    skv = nc.declare_dram_parameter("skv", [2, 3136], F32, isOutput=False)
    w1r = nc.declare_dram_parameter("w1r", [112, 28, 196], F32, isOutput=False)
    w2r = nc.declare_dram_parameter("w2r", [98, 2, 16], F32, isOutput=False)
    wt = nc.declare_dram_parameter("wt", [16, 2, 9, 128, 128], F32, isOutput=False)
    biash = nc.declare_dram_parameter("biash", [16, 128], F32, isOutput=False)
    selc = nc.declare_dram_parameter("selc", [2, 256], F32, isOutput=False)
    out2 = nc.declare_dram_parameter("out2", [2, 128, 64, 64], F32, isOutput=True)

    with TileContext(nc) as tc:
        with (
            tc.tile_pool(name="const", bufs=1) as cpool,
            tc.tile_pool(name="persist", bufs=1) as ppool,
            tc.tile_pool(name="wstream", bufs=6) as wpool,
            tc.tile_pool(name="xstage", bufs=2) as xpool,
            tc.tile_pool(name="outstage", bufs=8) as opool,
        ):
            # ---------------- small constant/param loads ----------------
            id_f32 = cpool.tile([16, 16], F32)
            make_identity(nc, id_f32)
            id_bf = cpool.tile([16, 16], BF16)
            make_identity(nc, id_bf)

            # sel[k, 128b:128b+128] = 1 if k == b else 0; used to broadcast
            # att row b across 128 partitions via PE.
            sel = cpool.tile([2, 256], F32)
            nc.sync.dma_start(out=sel, in_=selc[:])
            # DVE-copied twin so PE matmuls depend on a single engine (PE
            # instructions only support one sync wait).
            sel2 = cpool.tile([2, 256], F32)
            nc.vector.tensor_copy(sel2, sel)

            # Attention params stay fp32 and load via HWDGE (no SWDGE cast
            # dependency) so the attention chain finishes ASAP — it gates the
            # entire weight-mix stream.
            w1_sb = cpool.tile([112, 28, 196], F32)
            nc.sync.dma_start(out=w1_sb, in_=w1r[:])
            w2_sb = cpool.tile([98, 2, 16], F32)
            nc.sync.dma_start(out=w2_sb, in_=w2r[:])
            sk_sb = cpool.tile([112, 2, 28], F32)
            nc.sync.dma_start(
                out=sk_sb, in_=skv.rearrange("b (p c) -> p b c", p=112)
            )
            bias_sb = cpool.tile([16, 128], F32)
            nc.sync.dma_start(out=bias_sb, in_=biash[:])
            bias_sb2 = cpool.tile([16, 128], F32)
            nc.vector.tensor_copy(bias_sb2, bias_sb)

            att_bc = []  # [128, 16] f32 per sample: att[b, k] broadcast
            bias_b = ppool.tile([128, 2], F32)  # aggregated bias per sample

            with tc.tile_pool(name="psA", bufs=2, space="PSUM") as psA:
                # ---- hidden = relu(pooled @ w1.T) for both samples ----
                ps_h = psA.tile([2, 196], F32, tag="att_ps")
                for c in range(28):
                    nc.tensor.matmul(
                        ps_h,
                        sk_sb[:, :, c],          # [112, 2]
                        w1_sb[:, c, :],          # [112, 196]
                        start=(c == 0),
                        stop=(c == 27),
                    )
                hdn = ppool.tile([2, 196], F32)
                nc.vector.tensor_relu(hdn, ps_h)

                # transpose hdn chunks: [2, 98] -> [98, 2]
                hdnT = ppool.tile([98, 2, 2], F32)
                for c2 in range(2):
                    ps_t = psA.tile([98, 2], F32, tag="att_ps", name="ps_t")
                    nc.tensor.transpose(
                        ps_t, hdn[:, 98 * c2 : 98 * (c2 + 1)], id_f32[:2, :2]
                    )
                    nc.vector.tensor_copy(hdnT[:, c2, :], ps_t)

                # logits = hdn @ w2.T   -> [2, 16]
                ps_l = psA.tile([2, 16], F32, tag="att_ps", name="ps_l")
                for c2 in range(2):
                    nc.tensor.matmul(
                        ps_l,
                        hdnT[:, c2, :],          # [98, 2]
                        w2_sb[:, c2, :],         # [98, 16]
                        start=(c2 == 0),
                        stop=(c2 == 1),
                    )

                # softmax(logits / T) in f32 (ACT must stay off PSUM so conv
                # PSUM-bank reuse deps stay single-engine)
                logit_sb = ppool.tile([2, 16], F32)
                nc.vector.tensor_copy(logit_sb, ps_l)
                mx = ppool.tile([2, 1], F32)
                nc.vector.tensor_reduce(mx, logit_sb, axis=AX.X, op=OP.max)
                mxs = ppool.tile([2, 1], F32)
                nc.vector.tensor_scalar_mul(mxs, mx, -1.0 / TEMPERATURE)
                att_e = ppool.tile([2, 16], F32)
                nc.scalar.activation(
                    att_e, logit_sb, ACT.Exp, bias=mxs, scale=1.0 / TEMPERATURE
                )
                sm = ppool.tile([2, 1], F32)
                nc.vector.tensor_reduce(sm, att_e, axis=AX.X, op=OP.add)
                rec = ppool.tile([2, 1], F32)
                nc.vector.reciprocal(rec, sm)
                att_sb = ppool.tile([2, 16], F32)
                nc.vector.tensor_scalar_mul(att_sb, att_e, rec)

                # broadcast att rows across partitions: [128, 16] per sample
                for b in range(2):
                    ps_bc = psA.tile([128, 16], F32, tag="att_ps", name="ps_bc")
                    nc.tensor.matmul(
                        ps_bc, sel2[:, 128 * b : 128 * (b + 1)], att_sb,
                        start=True, stop=True,
                    )
                    abc = ppool.tile([128, 16], F32, name=f"att_bc{b}")
                    nc.vector.tensor_copy(abc, ps_bc)
                    att_bc.append(abc)

                # aggregated bias: bias_b[:, b] = sum_k att[b,k] bias[k, :]
                ps_at = psA.tile([16, 2], F32, tag="att_ps", name="ps_at")
                nc.tensor.transpose(ps_at, att_sb, id_f32[:2, :2])
                attT = ppool.tile([16, 2], F32)
                nc.vector.tensor_copy(attT, ps_at)
                ps_ab = psA.tile([128, 2], F32, tag="att_ps", name="ps_ab")
                nc.tensor.matmul(ps_ab, bias_sb2, attT, start=True, stop=True)
                nc.vector.tensor_copy(bias_b, ps_ab)

                # Age the att_bc/bias_b writes in the DVE stream: the first
                # weight-mix op below may carry only ONE sync wait (the slab
                # DMA), so its att_bc dep must be ≥ the DVE queue depth (8)
                # instructions old by the time it issues.
                age = ppool.tile([1, 16], F32, name="age")
                for j in range(8):
                    src = att_bc[j % 2] if j < 6 else bias_b
                    nc.vector.tensor_copy(age[:, j : j + 1], src[0:1, 0:1])

            # ---------------- x loads + zero-padded bf16 copies ----------------
            # xres[b]: f32 [128, 4096] for the residual (ci=0 == this core's
            # output-channel half, host-permuted).  xpad[b][ci]: bf16 [128,66,66].
            xres = []
            xpad = [[None, None], [None, None]]

            def load_x(b, ci):
                if ci == 0:
                    src_tile = ppool.tile([128, 4096], F32, name=f"xres{b}")
                    xres.append(src_tile)
                else:
                    src_tile = xpool.tile([128, 4096], F32, tag="xstg", name="xstg")
                nc.sync.dma_start(out=src_tile, in_=x2[b, 128 * ci : 128 * (ci + 1)])
                pad = ppool.tile([128, 66, 66], BF16, name=f"xpad{b}{ci}")
                nc.vector.memset(pad[:, 0, :], 0.0)
                nc.vector.memset(pad[:, 65, :], 0.0)
                nc.vector.memset(pad[:, 1:65, 0], 0.0)
                nc.vector.memset(pad[:, 1:65, 65], 0.0)
                # gpsimd (idle during the stream phase) does the cast-copy so
                # DVE stays dedicated to pacing the weight mix.
                nc.gpsimd.tensor_copy(
                    pad[:, 1:65, 1:65],
                    src_tile.rearrange("p (r c) -> p r c", r=64),
                )
                xpad[b][ci] = pad

            # ---------------- streamed weight aggregation ----------------
            # acc[b][ci][il, t, o] = sum_k att[b,k] * Wt[k, ci, t, il, o]
            acc = [[None, None], [None, None]]
            for b in range(2):
                for ci in range(2):
                    acc[b][ci] = ppool.tile([128, 9, 128], BF16, name=f"acc{b}{ci}")


            def stream_ci(ci):
                for b in range(2):
                    load_x(b, ci)
                for k in range(16):
                    wslab = wpool.tile([128, 9, 128], BF16, tag="wslab", name="wslab")
                    nc.gpsimd.dma_start(
                        out=wslab, in_=wt[k, ci].rearrange("t il o -> il t o")
                    )
                    # Weighted sum on DVE: TS-mul runs in 4x mode (360ns),
                    # TT-add in 2x mode (660ns).
                    for b in range(2):
                        if k == 0:
                            nc.vector.tensor_scalar_mul(
                                acc[b][ci], wslab, att_bc[b][:, 0:1]
                            )
                        else:
                            tmp = wpool.tile(
                                [128, 9, 128], BF16, tag="wtmp", name="wtmp", bufs=4
                            )
                            nc.vector.tensor_scalar_mul(
                                tmp, wslab, att_bc[b][:, k : k + 1]
                            )
                            nc.vector.tensor_add(acc[b][ci], acc[b][ci], tmp)

            stream_ci(0)
            stream_ci(1)

            # ---------------- conv + epilogue ----------------
            with tc.tile_pool(name="psC", bufs=8, space="PSUM") as psC:
                for blk in range(2):          # pixel-tile blocks: rows 0-31, 32-63
                    pts = range(4 * blk, 4 * blk + 4)
                    pcv = {}
                    # phase A: ci=0 taps (available early while ci=1 streams)
                    for b in range(2):
                        for pt in pts:
                            r0 = 8 * pt
                            p = psC.tile([128, 512], F32, tag="cv", name="pcv")
                            pcv[(b, pt)] = p
                            for t in range(9):
                                ty, tx = t // 3, t % 3
                                nc.tensor.matmul(
                                    p,
                                    acc[b][0][:, t, :],
                                    xpad[b][0][:, r0 + ty : r0 + ty + 8, tx : tx + 64],
                                    start=(t == 0),
                                    stop=False,
                                )
                    # phase B: ci=1 taps + epilogue
                    for b in range(2):
                        for pt in pts:
                            r0 = 8 * pt
                            p = pcv[(b, pt)]
                            for t in range(9):
                                ty, tx = t // 3, t % 3
                                nc.tensor.matmul(
                                    p,
                                    acc[b][1][:, t, :],
                                    xpad[b][1][:, r0 + ty : r0 + ty + 8, tx : tx + 64],
                                    start=False,
                                    stop=(t == 8),
                                )
                            osb = opool.tile([128, 512], F32, tag="osb", name="osb")
                            nc.scalar.activation(
                                osb, p, ACT.Identity, bias=bias_b[:, b : b + 1]
                            )
                            nc.gpsimd.tensor_tensor(
                                osb, osb, xres[b][:, 512 * pt : 512 * (pt + 1)], OP.add
                            )
                            nc.sync.dma_start(
                                out=out2[b, :, r0 : r0 + 8, :], in_=osb
                            )

    _split_multiwaits(nc)
    return nc


def _split_multiwaits(nc: bass.Bass):
    """This walrus build gives every TPB instruction exactly ONE sync-wait
    slot.  Tile emits multi-wait instructions; split the extras onto
    same-engine NoOp carriers inserted immediately before."""
    import bass_rust

    cnt = 0
    for fn in nc.m.functions:
        for blk in fn.blocks:
            out = []
            for ins in blk.instructions:
                si = getattr(ins, "sync_info", None)
                if si is not None and len(si.on_wait) > 1:
                    waits = list(si.on_wait)
                    for w in waits[:-1]:
                        cnt += 1
                        out.append(
                            bass_rust.InstNoOp(
                                name=f"waitcarrier-{cnt}",
                                engine=ins.engine,
                                ins=[],
                                outs=[],
                                sync_info=mybir.SyncInfo(
                                    on_wait=[w], on_update=[]
                                ),
                            )
                        )
                    ins.sync_info = mybir.SyncInfo(
                        on_wait=[waits[-1]], on_update=list(si.on_update)
                    )
                out.append(ins)
            blk.instructions = out


_PROGRAM = None


def _get_program():
    global _PROGRAM
    if _PROGRAM is None:
        _PROGRAM = build_program()
    return _PROGRAM


def _prepare_in_maps(x, scene_knowledge, weight, bias, att_w1, att_w2):
    x = np.ascontiguousarray(x, dtype=np.float32)
    scene_knowledge = np.ascontiguousarray(scene_knowledge, dtype=np.float32)
    weight = np.ascontiguousarray(weight, dtype=np.float32)
    bias = np.ascontiguousarray(bias, dtype=np.float32)
    att_w1 = np.ascontiguousarray(att_w1, dtype=np.float32)
    att_w2 = np.ascontiguousarray(att_w2, dtype=np.float32)

    K = 16
    # Wt[k, ci, t, il, o] = weight[k, o, 128*ci + il, ty*3+tx]
    Wt = np.ascontiguousarray(
        weight.reshape(K, 256, 2, 128, 9).transpose(0, 2, 4, 3, 1)
    )

    # fold 2x2 avg-pool into w1:  w1p[j, r*56+c] = 0.25 * w1[j, r//2, c//2]
    w1p = 0.25 * np.repeat(
        np.repeat(att_w1.reshape(196, 28, 28), 2, axis=1), 2, axis=2
    ).reshape(196, 3136)
    # w1r[p, c, j] = w1p[j, p*28 + c]
    w1r = np.ascontiguousarray(w1p.T.reshape(112, 28, 196))
    # w2r[p, c2, e] = att_w2[e, c2*98 + p]
    w2r = np.ascontiguousarray(att_w2.T.reshape(2, 98, 16).transpose(1, 0, 2))

    sel = np.zeros((2, 256), np.float32)
    sel[0, :128] = 1.0
    sel[1, 128:] = 1.0

    in_maps = []
    for c in range(NCORES):
        g, h = c // 2, c % 2
        perm = [h, 1 - h]  # i-chunk 0 == this core's output half (residual)
        x_core = np.ascontiguousarray(
            x[2 * g : 2 * g + 2].reshape(2, 2, 128, 64, 64)[:, perm]
        ).reshape(2, 256, 64, 64)
        wt_core = np.ascontiguousarray(
            Wt[:, perm][:, :, :, :, 128 * h : 128 * (h + 1)]
        )
        in_maps.append(
            {
                "x2": x_core,
                "skv": np.ascontiguousarray(
                    scene_knowledge[2 * g : 2 * g + 2].reshape(2, 3136)
                ),
                "w1r": w1r,
                "w2r": w2r,
                "wt": wt_core,
                "biash": np.ascontiguousarray(bias[:, 128 * h : 128 * (h + 1)]),
                "selc": sel,
            }
        )
    return in_maps


def _assemble(results):
    out = np.empty((8, 256, 64, 64), np.float32)
    for c in range(NCORES):
        g, h = c // 2, c % 2
        out[2 * g : 2 * g + 2, 128 * h : 128 * (h + 1)] = results[c]["out2"]
    return out


def run(inputs: dict, trace: bool = False, tmpdir: str | None = None):
    from concourse.bass_utils import run_bass_kernel_spmd

    nc = _get_program()
    in_maps = _prepare_in_maps(**inputs)
    res = run_bass_kernel_spmd(
        nc, in_maps, core_ids=list(range(NCORES)), trace=trace, tmpdir=tmpdir
    )
    return _assemble(res.results), res


def kernel(**inputs) -> np.ndarray:
    out, _ = run(inputs, trace=False)
    return out



# revision 3
# speedup vs baseline: 1.0899x; 1.0899x over previous
"""Trainium2 Bass kernel for the CondConv-style dense CNN.

Model (per sample b):
  att[b]  = softmax(MLP(avgpool(scene_knowledge[b])) / 30)        # [16]
  agg_w   = sum_k att[b,k] * weight[k]                            # [256,256,3,3]
  out[b]  = conv3x3_same(x[b], agg_w) + att[b] @ bias + x[b]

Sharding: 8 cores = 4 sample-pairs (g) x 2 output-channel halves (h).
Each core processes 2 samples and 128 output channels.

Device pipeline (everything bf16 except the attention chain + epilogue):
  1. attention chain -> att_bf[b] (bf16, broadcast across partitions)
  2. weight mix: acc[b][ci] += att[b,k] * wslab[k,ci] via fused DVE
     scalar_tensor_tensor ops, paced by the bf16 HWDGE slab stream
  3. residual folded into the conv: acc[b][0][:, center, :] += I
     (host permutes ci so chunk 0 == this core's output half)
  4. conv: 288 bf16 matmuls of [128c x 128p] x [128 x 512px]
  5. epilogue: ACT identity+bias -> f32 -> DMA out
"""

import sys
import numpy as np

sys.path.insert(0, "/opt/trn_rl_repo")

import ml_dtypes

import concourse.bass as bass
import concourse.mybir as mybir
from concourse.tile import TileContext
from concourse.masks import make_identity

F32 = mybir.dt.float32
BF16 = mybir.dt.bfloat16
AX = mybir.AxisListType
OP = mybir.AluOpType
ACT = mybir.ActivationFunctionType

TEMPERATURE = 30.0
NCORES = 8
BF = ml_dtypes.bfloat16


def build_program() -> bass.Bass:
    nc = bass.Bass()

    xp = nc.declare_dram_parameter("xp", [2, 2, 128, 66, 66], BF16, isOutput=False)
    skv = nc.declare_dram_parameter("skv", [2, 3136], F32, isOutput=False)
    w1r = nc.declare_dram_parameter("w1r", [112, 28, 196], F32, isOutput=False)
    w2r = nc.declare_dram_parameter("w2r", [98, 2, 16], F32, isOutput=False)
    wt = nc.declare_dram_parameter("wt", [16, 2, 128, 9, 128], BF16, isOutput=False)
    biash = nc.declare_dram_parameter("biash", [16, 128], F32, isOutput=False)
    selc = nc.declare_dram_parameter("selc", [2, 256], F32, isOutput=False)
    idm = nc.declare_dram_parameter("idm", [128, 128], BF16, isOutput=False)
    out2 = nc.declare_dram_parameter("out2", [2, 128, 64, 64], F32, isOutput=True)

    with TileContext(nc) as tc:
        with (
            tc.tile_pool(name="const", bufs=1) as cpool,
            tc.tile_pool(name="persist", bufs=1) as ppool,
            tc.tile_pool(name="wstream", bufs=6) as wpool,
            tc.tile_pool(name="outstage", bufs=8) as opool,
        ):
            # ---------------- small constant/param loads ----------------
            id_f32 = cpool.tile([16, 16], F32)
            make_identity(nc, id_f32)

            sel = cpool.tile([2, 256], F32)
            nc.sync.dma_start(out=sel, in_=selc[:])
            sel2 = cpool.tile([2, 256], F32)
            nc.vector.tensor_copy(sel2, sel)

            id_sb = cpool.tile([128, 128], BF16)
            nc.sync.dma_start(out=id_sb, in_=idm[:])

            w1_sb = cpool.tile([112, 28, 196], F32)
            nc.sync.dma_start(out=w1_sb, in_=w1r[:])
            w2_sb = cpool.tile([98, 2, 16], F32)
            nc.sync.dma_start(out=w2_sb, in_=w2r[:])
            sk_sb = cpool.tile([112, 2, 28], F32)
            nc.sync.dma_start(
                out=sk_sb, in_=skv.rearrange("b (p c) -> p b c", p=112)
            )
            bias_sb = cpool.tile([16, 128], F32)
            nc.sync.dma_start(out=bias_sb, in_=biash[:])
            bias_sb2 = cpool.tile([16, 128], F32)
            nc.vector.tensor_copy(bias_sb2, bias_sb)

            att_bf = []  # [128, 16] f32 per sample: att[b, k] broadcast
            bias_b = ppool.tile([128, 2], F32)  # aggregated bias per sample

            with tc.tile_pool(name="psA", bufs=2, space="PSUM") as psA:
                # ---- hidden = relu(pooled @ w1.T) for both samples ----
                ps_h = psA.tile([2, 196], F32, tag="att_ps")
                for c in range(28):
                    nc.tensor.matmul(
                        ps_h,
                        sk_sb[:, :, c],          # [112, 2]
                        w1_sb[:, c, :],          # [112, 196]
                        start=(c == 0),
                        stop=(c == 27),
                    )
                hdn = ppool.tile([2, 196], F32)
                nc.vector.tensor_relu(hdn, ps_h)

                # transpose hdn chunks: [2, 98] -> [98, 2]
                hdnT = ppool.tile([98, 2, 2], F32)
                for c2 in range(2):
                    ps_t = psA.tile([98, 2], F32, tag="att_ps", name="ps_t")
                    nc.tensor.transpose(
                        ps_t, hdn[:, 98 * c2 : 98 * (c2 + 1)], id_f32[:2, :2]
                    )
                    nc.vector.tensor_copy(hdnT[:, c2, :], ps_t)

                # logits = hdn @ w2.T   -> [2, 16]
                ps_l = psA.tile([2, 16], F32, tag="att_ps", name="ps_l")
                for c2 in range(2):
                    nc.tensor.matmul(
                        ps_l,
                        hdnT[:, c2, :],          # [98, 2]
                        w2_sb[:, c2, :],         # [98, 16]
                        start=(c2 == 0),
                        stop=(c2 == 1),
                    )

                # softmax(logits / T) in f32
                logit_sb = ppool.tile([2, 16], F32)
                nc.vector.tensor_copy(logit_sb, ps_l)
                mx = ppool.tile([2, 1], F32)
                nc.vector.tensor_reduce(mx, logit_sb, axis=AX.X, op=OP.max)
                mxs = ppool.tile([2, 1], F32)
                nc.vector.tensor_scalar_mul(mxs, mx, -1.0 / TEMPERATURE)
                att_e = ppool.tile([2, 16], F32)
                nc.scalar.activation(
                    att_e, logit_sb, ACT.Exp, bias=mxs, scale=1.0 / TEMPERATURE
                )
                sm = ppool.tile([2, 1], F32)
                nc.vector.tensor_reduce(sm, att_e, axis=AX.X, op=OP.add)
                rec = ppool.tile([2, 1], F32)
                nc.vector.reciprocal(rec, sm)
                att_sb = ppool.tile([2, 16], F32)
                nc.vector.tensor_scalar_mul(att_sb, att_e, rec)

                # broadcast att rows across partitions: [128, 16] per sample
                for b in range(2):
                    ps_bc = psA.tile([128, 16], F32, tag="att_ps", name="ps_bc")
                    nc.tensor.matmul(
                        ps_bc, sel2[:, 128 * b : 128 * (b + 1)], att_sb,
                        start=True, stop=True,
                    )
                    abf = ppool.tile([128, 16], F32, name=f"att_bc{b}")
                    nc.vector.tensor_copy(abf, ps_bc)
                    att_bf.append(abf)

                # aggregated bias: bias_b[:, b] = sum_k att[b,k] bias[k, :]
                ps_at = psA.tile([16, 2], F32, tag="att_ps", name="ps_at")
                nc.tensor.transpose(ps_at, att_sb, id_f32[:2, :2])
                attT = ppool.tile([16, 2], F32)
                nc.vector.tensor_copy(attT, ps_at)
                ps_ab = psA.tile([128, 2], F32, tag="att_ps", name="ps_ab")
                nc.tensor.matmul(ps_ab, bias_sb2, attT, start=True, stop=True)
                nc.vector.tensor_copy(bias_b, ps_ab)

                # Age the att_bf/bias_b writes in the DVE stream so the first
                # mix op's same-engine dep is safely covered by queue depth.
                age = ppool.tile([1, 16], F32, name="age")
                for j in range(8):
                    src = att_bf[j % 2] if j < 6 else bias_b
                    nc.vector.tensor_copy(age[:, j : j + 1], src[0:1, 0:1])

            # ---------------- x loads (pre-padded bf16 from host) ----------------
            xpad = [[None, None], [None, None]]
            for b in range(2):
                for ci in range(2):
                    pad = ppool.tile([128, 66, 66], BF16, name=f"xpad{b}{ci}")
                    nc.scalar.dma_start(out=pad, in_=xp[b, ci])
                    xpad[b][ci] = pad

            # ---------------- streamed weight aggregation ----------------
            # acc[b][ci][il, t, o] = sum_k att[b,k] * Wt[k, ci, il, t, o]
            acc = [[None, None], [None, None]]
            for b in range(2):
                for ci in range(2):
                    acc[b][ci] = ppool.tile([128, 9, 128], BF16, name=f"acc{b}{ci}")

            with tc.tile_pool(name="psW", bufs=1, space="PSUM") as psW:
                for ci in range(2):
                    for k in range(16):
                        wslab = wpool.tile(
                            [128, 9, 128], BF16, tag="wslab", name="wslab"
                        )
                        nc.sync.dma_start(out=wslab, in_=wt[k, ci])
                        for b in range(2):
                            sc = att_bf[b][:, k : k + 1]
                            if k == 0:
                                nc.vector.tensor_scalar_mul(acc[b][ci], wslab, sc)
                            else:
                                nc.vector.scalar_tensor_tensor(
                                    acc[b][ci], wslab, sc, acc[b][ci],
                                    op0=OP.mult, op1=OP.add,
                                )
                        if ci == 0:
                            # keep-warm matmul paced by the slab stream so the
                            # PE's HAM clock gate stays at full rate.
                            pw = psW.tile([128, 512], F32, tag="warm", name="pw")
                            nc.tensor.matmul(
                                pw, wslab[:, 0, :], wslab[:, 1:5, :],
                                start=True, stop=True,
                            )

            # residual as conv identity tap: acc[b][0][:, center, :] += I
            # (ci chunk 0 is host-permuted to this core's output half)
            for b in range(2):
                nc.vector.tensor_add(
                    acc[b][0][:, 4, :], acc[b][0][:, 4, :], id_sb
                )

            # ---------------- conv + epilogue ----------------
            with tc.tile_pool(name="psC", bufs=8, space="PSUM") as psC:
                for blk in range(2):          # pixel-tile blocks: rows 0-31, 32-63
                    pts = range(4 * blk, 4 * blk + 4)
                    pcv = {}
                    # phase A: ci=0 taps (available early while ci=1 streams)
                    for b in range(2):
                        for pt in pts:
                            r0 = 8 * pt
                            p = psC.tile([128, 512], F32, tag="cv", name="pcv")
                            pcv[(b, pt)] = p
                            for t in range(9):
                                ty, tx = t // 3, t % 3
                                nc.tensor.matmul(
                                    p,
                                    acc[b][0][:, t, :],
                                    xpad[b][0][:, r0 + ty : r0 + ty + 8, tx : tx + 64],
                                    start=(t == 0),
                                    stop=False,
                                )
                    # phase B: ci=1 taps + epilogue
                    for b in range(2):
                        for pt in pts:
                            r0 = 8 * pt
                            p = pcv[(b, pt)]
                            for t in range(9):
                                ty, tx = t // 3, t % 3
                                nc.tensor.matmul(
                                    p,
                                    acc[b][1][:, t, :],
                                    xpad[b][1][:, r0 + ty : r0 + ty + 8, tx : tx + 64],
                                    start=False,
                                    stop=(t == 8),
                                )
                            osb = opool.tile([128, 512], F32, tag="osb", name="osb")
                            nc.scalar.activation(
                                osb, p, ACT.Identity, bias=bias_b[:, b : b + 1]
                            )
                            nc.sync.dma_start(
                                out=out2[b, :, r0 : r0 + 8, :], in_=osb
                            )

    _split_multiwaits(nc)
    return nc


def _split_multiwaits(nc: bass.Bass):
    """This walrus build gives every TPB instruction exactly ONE sync-wait
    slot.  Tile emits multi-wait instructions; split the extras onto
    same-engine NoOp carriers inserted immediately before."""
    import bass_rust

    cnt = 0
    for fn in nc.m.functions:
        for blk in fn.blocks:
            out = []
            for ins in blk.instructions:
                si = getattr(ins, "sync_info", None)
                if si is not None and len(si.on_wait) > 1:
                    waits = list(si.on_wait)
                    for w in waits[:-1]:
                        cnt += 1
                        out.append(
                            bass_rust.InstNoOp(
                                name=f"waitcarrier-{cnt}",
                                engine=ins.engine,
                                ins=[],
                                outs=[],
                                sync_info=mybir.SyncInfo(
                                    on_wait=[w], on_update=[]
                                ),
                            )
                        )
                    ins.sync_info = mybir.SyncInfo(
                        on_wait=[waits[-1]], on_update=list(si.on_update)
                    )
                out.append(ins)
            blk.instructions = out


_PROGRAM = None


def _get_program():
    global _PROGRAM
    if _PROGRAM is None:
        _PROGRAM = build_program()
    return _PROGRAM


def _prepare_in_maps(x, scene_knowledge, weight, bias, att_w1, att_w2):
    x = np.ascontiguousarray(x, dtype=np.float32)
    scene_knowledge = np.ascontiguousarray(scene_knowledge, dtype=np.float32)
    weight = np.ascontiguousarray(weight, dtype=np.float32)
    bias = np.ascontiguousarray(bias, dtype=np.float32)
    att_w1 = np.ascontiguousarray(att_w1, dtype=np.float32)
    att_w2 = np.ascontiguousarray(att_w2, dtype=np.float32)

    K = 16
    # Wt[k, h, ci, il, t, o] = weight[k, 128h+o, 128ci+il, ty, tx]
    Wt = (
        weight.reshape(K, 2, 128, 2, 128, 3, 3)
        .transpose(0, 1, 3, 4, 5, 6, 2)
        .astype(BF)
        .reshape(K, 2, 2, 128, 9, 128)
    )

    # x zero-padded to 66x66 in bf16: [bs, ci, il, 66, 66]
    xpad = np.zeros((8, 2, 128, 66, 66), BF)
    xpad[:, :, :, 1:65, 1:65] = x.reshape(8, 2, 128, 64, 64)

    # fold 2x2 avg-pool into w1:  w1p[j, r*56+c] = 0.25 * w1[j, r//2, c//2]
    w1p = 0.25 * np.repeat(
        np.repeat(att_w1.reshape(196, 28, 28), 2, axis=1), 2, axis=2
    ).reshape(196, 3136)
    # w1r[p, c, j] = w1p[j, p*28 + c]
    w1r = np.ascontiguousarray(w1p.T.reshape(112, 28, 196))
    # w2r[p, c2, e] = att_w2[e, c2*98 + p]
    w2r = np.ascontiguousarray(att_w2.T.reshape(2, 98, 16).transpose(1, 0, 2))

    sel = np.zeros((2, 256), np.float32)
    sel[0, :128] = 1.0
    sel[1, 128:] = 1.0

    idm = np.eye(128, dtype=np.float32).astype(BF)

    in_maps = []
    for c in range(NCORES):
        g, h = c // 2, c % 2
        perm = [h, 1 - h]  # ci chunk 0 == this core's output half
        in_maps.append(
            {
                "xp": np.ascontiguousarray(xpad[2 * g : 2 * g + 2][:, perm]),
                "skv": np.ascontiguousarray(
                    scene_knowledge[2 * g : 2 * g + 2].reshape(2, 3136)
                ),
                "w1r": w1r,
                "w2r": w2r,
                "wt": np.ascontiguousarray(Wt[:, h][:, perm]),
                "biash": np.ascontiguousarray(bias[:, 128 * h : 128 * (h + 1)]),
                "selc": sel,
                "idm": idm,
            }
        )
    return in_maps


def _assemble(results):
    out = np.empty((8, 256, 64, 64), np.float32)
    for c in range(NCORES):
        g, h = c // 2, c % 2
        out[2 * g : 2 * g + 2, 128 * h : 128 * (h + 1)] = results[c]["out2"]
    return out


def run(inputs: dict, trace: bool = False, tmpdir: str | None = None):
    from concourse.bass_utils import run_bass_kernel_spmd

    nc = _get_program()
    in_maps = _prepare_in_maps(**inputs)
    res = run_bass_kernel_spmd(
        nc, in_maps, core_ids=list(range(NCORES)), trace=trace, tmpdir=tmpdir
    )
    return _assemble(res.results), res


def kernel(**inputs) -> np.ndarray:
    out, _ = run(inputs, trace=False)
    return out


# revision 5
# speedup vs baseline: 1.5068x; 1.3825x over previous
"""Trainium2 Bass kernel for the CondConv-style dense CNN.

Model (per sample b):
  att[b]  = softmax(MLP(avgpool(scene_knowledge[b])) / 30)        # [16]
  agg_w   = sum_k att[b,k] * weight[k]                            # [256,256,3,3]
  out[b]  = conv3x3_same(x[b], agg_w) + att[b] @ bias + x[b]

Sharding: 8 cores = 4 sample-pairs (g) x 2 output-channel halves (h).
Each core processes 2 samples and 128 output channels.

The expert mix runs on the PE as selector matmuls: stationary operand is a
weight chunk [(o8,k)=128, il=128] (o8 = output channel mod-8 block, k =
expert), moving operand is a [128, 16] selector rhs_sel[(o8,k),(b,o8')] =
att[b,k] * delta(o8,o8'), so each chunk matmul emits aggregated weights
for 8 output channels x both samples straight into PSUM with partition =
il — the exact lhsT orientation the conv needs.  DVE only drains PSUM to
bf16 SBUF.  The residual is folded into the conv as a +I center tap.
"""

import sys
import numpy as np

sys.path.insert(0, "/opt/trn_rl_repo")

import ml_dtypes

import concourse.bass as bass
import concourse.mybir as mybir
from concourse.tile import TileContext
from concourse.masks import make_identity

F32 = mybir.dt.float32
BF16 = mybir.dt.bfloat16
AX = mybir.AxisListType
OP = mybir.AluOpType
ACT = mybir.ActivationFunctionType

TEMPERATURE = 30.0
NCORES = 8
BF = ml_dtypes.bfloat16

# ob groups per PSUM bank during the mix: [3, 3, 3, 3, 3, 1]
OB_GROUPS = [(0, 3), (3, 3), (6, 3), (9, 3), (12, 3), (15, 1)]


def build_program() -> bass.Bass:
    nc = bass.Bass()

    xp = nc.declare_dram_parameter("xp", [2, 2, 128, 66, 66], BF16, isOutput=False)
    skb = nc.declare_dram_parameter("skb", [112, 2, 28], BF16, isOutput=False)
    w1b = nc.declare_dram_parameter("w1b", [112, 28, 196], BF16, isOutput=False)
    w2b = nc.declare_dram_parameter("w2b", [98, 2, 16], BF16, isOutput=False)
    wm = nc.declare_dram_parameter("wm", [2, 16, 128, 9, 128], BF16, isOutput=False)
    biash = nc.declare_dram_parameter("biash", [16, 128], F32, isOutput=False)
    selk = nc.declare_dram_parameter("selk", [16, 128], F32, isOutput=False)
    mask8 = nc.declare_dram_parameter("mask8", [128, 8], BF16, isOutput=False)
    idm = nc.declare_dram_parameter("idm", [128, 128], BF16, isOutput=False)
    out2 = nc.declare_dram_parameter("out2", [2, 128, 64, 64], F32, isOutput=True)

    with TileContext(nc) as tc:
        with (
            tc.tile_pool(name="const", bufs=1) as cpool,
            tc.tile_pool(name="persist", bufs=1) as ppool,
            tc.tile_pool(name="wstream", bufs=6) as wpool,
            tc.tile_pool(name="outstage", bufs=8) as opool,
        ):
            # ---------------- small constant/param loads ----------------
            id_f32 = cpool.tile([16, 16], F32)
            make_identity(nc, id_f32)
            id_bf = cpool.tile([16, 16], BF16)
            make_identity(nc, id_bf)

            # attention params first on the sync queue (small, gate the mix)
            sk_sb = cpool.tile([112, 2, 28], BF16)
            nc.sync.dma_start(out=sk_sb, in_=skb[:])
            w2_sb = cpool.tile([98, 2, 16], BF16)
            nc.sync.dma_start(out=w2_sb, in_=w2b[:])
            selk_sb = cpool.tile([16, 128], F32)
            nc.sync.dma_start(out=selk_sb, in_=selk[:])
            mask_sb = cpool.tile([128, 8], BF16)
            nc.sync.dma_start(out=mask_sb, in_=mask8[:])
            bias_sb = cpool.tile([16, 128], F32)
            nc.sync.dma_start(out=bias_sb, in_=biash[:])
            id_sb = cpool.tile([128, 128], BF16)
            nc.sync.dma_start(out=id_sb, in_=idm[:])
            w1_sb = cpool.tile([112, 28, 196], BF16)
            nc.sync.dma_start(out=w1_sb, in_=w1b[:])

            bias_b = ppool.tile([128, 2], F32)   # aggregated bias per sample
            rhs_sel = ppool.tile([128, 16], BF16)  # att[b,k]*d(o8,o8')

            with tc.tile_pool(name="psA", bufs=2, space="PSUM") as psA:
                # ---- hidden = relu(pooled @ w1.T) for both samples ----
                ps_h = psA.tile([2, 196], F32, tag="att_ps")
                for c in range(28):
                    nc.tensor.matmul(
                        ps_h,
                        sk_sb[:, :, c],          # [112, 2]
                        w1_sb[:, c, :],          # [112, 196]
                        start=(c == 0),
                        stop=(c == 27),
                    )
                hdn = ppool.tile([2, 196], BF16)
                nc.vector.tensor_relu(hdn, ps_h)

                # transpose hdn chunks: [2, 98] -> [98, 2]
                hdnT = ppool.tile([98, 2, 2], BF16)
                for c2 in range(2):
                    ps_t = psA.tile([98, 2], BF16, tag="att_ps", name="ps_t")
                    nc.tensor.transpose(
                        ps_t, hdn[:, 98 * c2 : 98 * (c2 + 1)], id_bf[:2, :2]
                    )
                    nc.vector.tensor_copy(hdnT[:, c2, :], ps_t)

                # logits = hdn @ w2.T   -> [2, 16]
                ps_l = psA.tile([2, 16], F32, tag="att_ps", name="ps_l")
                for c2 in range(2):
                    nc.tensor.matmul(
                        ps_l,
                        hdnT[:, c2, :],          # [98, 2]
                        w2_sb[:, c2, :],         # [98, 16]
                        start=(c2 == 0),
                        stop=(c2 == 1),
                    )

                # softmax(logits / T) in f32
                logit_sb = ppool.tile([2, 16], F32)
                nc.vector.tensor_copy(logit_sb, ps_l)
                mx = ppool.tile([2, 1], F32)
                nc.vector.tensor_reduce(mx, logit_sb, axis=AX.X, op=OP.max)
                mxs = ppool.tile([2, 1], F32)
                nc.vector.tensor_scalar_mul(mxs, mx, -1.0 / TEMPERATURE)
                att_e = ppool.tile([2, 16], F32)
                nc.scalar.activation(
                    att_e, logit_sb, ACT.Exp, bias=mxs, scale=1.0 / TEMPERATURE
                )
                sm = ppool.tile([2, 1], F32)
                nc.vector.tensor_reduce(sm, att_e, axis=AX.X, op=OP.add)
                rec = ppool.tile([2, 1], F32)
                nc.vector.reciprocal(rec, sm)
                att_sb = ppool.tile([2, 16], F32)
                nc.vector.tensor_scalar_mul(att_sb, att_e, rec)

                # attT[k, b] = att[b, k]
                ps_at = psA.tile([16, 2], F32, tag="att_ps", name="ps_at")
                nc.tensor.transpose(ps_at, att_sb, id_f32[:2, :2])
                attT = ppool.tile([16, 2], F32)
                nc.vector.tensor_copy(attT, ps_at)

                # aggregated bias: bias_b[:, b] = sum_k att[b,k] bias[k, :]
                ps_ab = psA.tile([128, 2], F32, tag="att_ps", name="ps_ab")
                nc.tensor.matmul(ps_ab, bias_sb, attT, start=True, stop=True)
                nc.vector.tensor_copy(bias_b, ps_ab)

                # attc[(o8,k), b] = att[b, k]  (replicated over o8)
                ps_ac = psA.tile([128, 2], F32, tag="att_ps", name="ps_ac")
                nc.tensor.matmul(ps_ac, selk_sb, attT, start=True, stop=True)
                attc = ppool.tile([128, 2], F32)
                nc.vector.tensor_copy(attc, ps_ac)

                # rhs_sel[(o8,k), (b,o8')] = att[b,k] * d(o8,o8')
                for b in range(2):
                    nc.vector.tensor_scalar_mul(
                        rhs_sel[:, 8 * b : 8 * b + 8], mask_sb, attc[:, b : b + 1]
                    )

            # ---------------- x loads (pre-padded bf16 from host) ----------------
            xpad = [[None, None], [None, None]]
            for b in range(2):
                for ci in range(2):
                    pad = ppool.tile([128, 66, 66], BF16, name=f"xpad{b}{ci}")
                    nc.scalar.dma_start(out=pad, in_=xp[b, ci])
                    xpad[b][ci] = pad

            # ---------------- PE weight mix ----------------
            # acc[b][ci][il, t, o] = sum_k att[b,k] * W[k, o(half), 128ci+il, t]
            acc = [[None, None], [None, None]]
            for b in range(2):
                for ci in range(2):
                    acc[b][ci] = ppool.tile([128, 9, 128], BF16, name=f"acc{b}{ci}")

            with (
                tc.tile_pool(name="psM", bufs=2, space="PSUM") as psM,
                tc.tile_pool(name="psC", bufs=6, space="PSUM") as psC,
            ):
                for ci in range(2):
                    for g, (ob0, nob) in enumerate(OB_GROUPS):
                        pm = psM.tile([128, 512], F32, tag="mix", name="pm")
                        for ol in range(nob):
                            wslab = wpool.tile(
                                [128, 9, 128], BF16, tag="wslab", name="wslab"
                            )
                            nc.sync.dma_start(out=wslab, in_=wm[ci, ob0 + ol])
                            for t in range(9):
                                col = ol * 144 + t * 16
                                nc.tensor.matmul(
                                    pm[:, col : col + 16],
                                    wslab[:, t, :],
                                    rhs_sel,
                                    start=True,
                                    stop=True,
                                )
                        # drain the bank: [128, (ol, t, b, o8)] -> acc[b][ci]
                        view = pm[:, : nob * 144].rearrange(
                            "p (ol t e) -> p t ol e", ol=nob, t=9, e=16
                        )
                        for b in range(2):
                            nc.vector.tensor_copy(
                                acc[b][ci][:, :, 8 * ob0 : 8 * (ob0 + nob)],
                                view[:, :, :, 8 * b : 8 * b + 8],
                            )
                    if ci == 0:
                        # residual as conv identity tap on this core's own half
                        for b in range(2):
                            nc.vector.tensor_add(
                                acc[b][0][:, 4, :], acc[b][0][:, 4, :], id_sb
                            )

                # ---------------- conv + epilogue ----------------
                tiles = [(b, pt) for b in range(2) for pt in range(8)]
                for w0 in range(0, 16, 6):          # waves of 6 psum banks
                    wave = tiles[w0 : w0 + 6]
                    pcv = {}
                    for b, pt in wave:              # phase A: ci=0 taps
                        r0 = 8 * pt
                        p = psC.tile([128, 512], F32, tag="cv", name="pcv")
                        pcv[(b, pt)] = p
                        for t in range(9):
                            ty, tx = t // 3, t % 3
                            nc.tensor.matmul(
                                p,
                                acc[b][0][:, t, :],
                                xpad[b][0][:, r0 + ty : r0 + ty + 8, tx : tx + 64],
                                start=(t == 0),
                                stop=False,
                            )
                    for b, pt in wave:              # phase B: ci=1 + epilogue
                        r0 = 8 * pt
                        p = pcv[(b, pt)]
                        for t in range(9):
                            ty, tx = t // 3, t % 3
                            nc.tensor.matmul(
                                p,
                                acc[b][1][:, t, :],
                                xpad[b][1][:, r0 + ty : r0 + ty + 8, tx : tx + 64],
                                start=False,
                                stop=(t == 8),
                            )
                        osb = opool.tile([128, 512], F32, tag="osb", name="osb")
                        nc.scalar.activation(
                            osb, p, ACT.Identity, bias=bias_b[:, b : b + 1]
                        )
                        nc.sync.dma_start(
                            out=out2[b, :, r0 : r0 + 8, :], in_=osb
                        )

    _split_multiwaits(nc)
    return nc


def _split_multiwaits(nc: bass.Bass):
    """This walrus build gives every TPB instruction exactly ONE sync-wait
    slot.  Tile emits multi-wait instructions; split the extras onto
    same-engine NoOp carriers inserted immediately before."""
    import bass_rust

    cnt = 0
    for fn in nc.m.functions:
        for blk in fn.blocks:
            out = []
            for ins in blk.instructions:
                si = getattr(ins, "sync_info", None)
                if si is not None and len(si.on_wait) > 1:
                    waits = list(si.on_wait)
                    for w in waits[:-1]:
                        cnt += 1
                        out.append(
                            bass_rust.InstNoOp(
                                name=f"waitcarrier-{cnt}",
                                engine=ins.engine,
                                ins=[],
                                outs=[],
                                sync_info=mybir.SyncInfo(
                                    on_wait=[w], on_update=[]
                                ),
                            )
                        )
                    ins.sync_info = mybir.SyncInfo(
                        on_wait=[waits[-1]], on_update=list(si.on_update)
                    )
                out.append(ins)
            blk.instructions = out


_PROGRAM = None


def _get_program():
    global _PROGRAM
    if _PROGRAM is None:
        _PROGRAM = build_program()
    return _PROGRAM


def _prepare_in_maps(x, scene_knowledge, weight, bias, att_w1, att_w2):
    x = np.ascontiguousarray(x, dtype=np.float32)
    scene_knowledge = np.ascontiguousarray(scene_knowledge, dtype=np.float32)
    weight = np.ascontiguousarray(weight, dtype=np.float32)
    bias = np.ascontiguousarray(bias, dtype=np.float32)
    att_w1 = np.ascontiguousarray(att_w1, dtype=np.float32)
    att_w2 = np.ascontiguousarray(att_w2, dtype=np.float32)

    K = 16
    # Wm[h, ci, ob, (o8,k), t, il] = weight[k, 128h+8ob+o8, 128ci+il, ty, tx]
    Wm = (
        weight.reshape(K, 2, 16, 8, 2, 128, 3, 3)  # k h ob o8 ci il ty tx
        .transpose(1, 4, 2, 3, 0, 6, 7, 5)         # h ci ob o8 k ty tx il
        .astype(BF)
        .reshape(2, 2, 16, 128, 9, 128)
    )

    # x zero-padded to 66x66 in bf16: [bs, ci, il, 66, 66]
    xpad = np.zeros((8, 2, 128, 66, 66), BF)
    xpad[:, :, :, 1:65, 1:65] = x.reshape(8, 2, 128, 64, 64)

    # fold 2x2 avg-pool into w1:  w1p[j, r*56+c] = 0.25 * w1[j, r//2, c//2]
    w1p = 0.25 * np.repeat(
        np.repeat(att_w1.reshape(196, 28, 28), 2, axis=1), 2, axis=2
    ).reshape(196, 3136)
    # w1b[p, c, j] = w1p[j, p*28 + c]
    w1bf = np.ascontiguousarray(w1p.T.reshape(112, 28, 196).astype(BF))
    # w2b[p, c2, e] = att_w2[e, c2*98 + p]
    w2bf = np.ascontiguousarray(
        att_w2.T.reshape(2, 98, 16).transpose(1, 0, 2).astype(BF)
    )
    # skb[p, b, c] = scene[b, p*28 + c]
    skbf = np.ascontiguousarray(
        scene_knowledge.reshape(8, 3136).reshape(8, 112, 28).transpose(1, 0, 2)
    ).astype(BF)

    p128 = np.arange(128)
    selk = np.zeros((16, 128), np.float32)
    selk[p128 % 16, p128] = 1.0
    mask8 = np.zeros((128, 8), np.float32)
    mask8[p128, p128 // 16] = 1.0
    mask8 = mask8.astype(BF)
    idm = np.eye(128, dtype=np.float32).astype(BF)

    in_maps = []
    for c in range(NCORES):
        g, h = c // 2, c % 2
        perm = [h, 1 - h]  # ci chunk 0 == this core's output half
        in_maps.append(
            {
                "xp": np.ascontiguousarray(xpad[2 * g : 2 * g + 2][:, perm]),
                "skb": np.ascontiguousarray(skbf[:, 2 * g : 2 * g + 2]),
                "w1b": w1bf,
                "w2b": w2bf,
                "wm": np.ascontiguousarray(Wm[h][perm]),
                "biash": np.ascontiguousarray(bias[:, 128 * h : 128 * (h + 1)]),
                "selk": selk,
                "mask8": mask8,
                "idm": idm,
            }
        )
    return in_maps


def _assemble(results):
    out = np.empty((8, 256, 64, 64), np.float32)
    for c in range(NCORES):
        g, h = c // 2, c % 2
        out[2 * g : 2 * g + 2, 128 * h : 128 * (h + 1)] = results[c]["out2"]
    return out


def run(inputs: dict, trace: bool = False, tmpdir: str | None = None):
    from concourse.bass_utils import run_bass_kernel_spmd

    nc = _get_program()
    in_maps = _prepare_in_maps(**inputs)
    res = run_bass_kernel_spmd(
        nc, in_maps, core_ids=list(range(NCORES)), trace=trace, tmpdir=tmpdir
    )
    return _assemble(res.results), res


def kernel(**inputs) -> np.ndarray:
    out, _ = run(inputs, trace=False)
    return out


# revision 8
# speedup vs baseline: 1.6034x; 1.0641x over previous
"""Trainium2 Bass kernel for the CondConv-style dense CNN.

Model (per sample b):
  att[b]  = softmax(MLP(avgpool(scene_knowledge[b])) / 30)        # [16]
  agg_w   = sum_k att[b,k] * weight[k]                            # [256,256,3,3]
  out[b]  = conv3x3_same(x[b], agg_w) + att[b] @ bias + x[b]

Sharding: 8 cores = 4 sample-pairs (g) x 2 output-channel halves (h).
Each core processes 2 samples and 128 output channels.

The expert mix runs on the PE as selector matmuls: stationary operand is a
weight chunk [(o8,k)=128, il=128] (o8 = output channel mod-8 block, k =
expert), moving operand is a [128, 16] selector rhs_sel[(o8,k),(b,o8')] =
att[b,k] * delta(o8,o8'), so each chunk matmul emits aggregated weights
for 8 output channels x both samples straight into PSUM with partition =
il — the exact lhsT orientation the conv needs.  DVE only drains PSUM to
bf16 SBUF.  The residual is folded into the conv as a +I center tap.
"""

import sys
import numpy as np

sys.path.insert(0, "/opt/trn_rl_repo")

import ml_dtypes

import concourse.bass as bass
import concourse.mybir as mybir
from concourse.tile import TileContext
from concourse.masks import make_identity

F32 = mybir.dt.float32
BF16 = mybir.dt.bfloat16
AX = mybir.AxisListType
OP = mybir.AluOpType
ACT = mybir.ActivationFunctionType

TEMPERATURE = 30.0
NCORES = 8
BF = ml_dtypes.bfloat16

# ob groups per PSUM bank during the mix: [3, 3, 3, 3, 3, 1]
OB_GROUPS = [(0, 3), (3, 3), (6, 3), (9, 3), (12, 3), (15, 1)]


def build_program() -> bass.Bass:
    nc = bass.Bass()

    xp = nc.declare_dram_parameter("xp", [2, 2, 128, 66, 66], BF16, isOutput=False)
    skb = nc.declare_dram_parameter("skb", [112, 2, 28], BF16, isOutput=False)
    w1b = nc.declare_dram_parameter("w1b", [2, 112, 14, 196], BF16, isOutput=False)
    w2b = nc.declare_dram_parameter("w2b", [98, 2, 16], BF16, isOutput=False)
    wm = nc.declare_dram_parameter("wm", [2, 16, 128, 9, 128], BF16, isOutput=False)
    biash = nc.declare_dram_parameter("biash", [16, 128], F32, isOutput=False)
    selk = nc.declare_dram_parameter("selk", [16, 128], F32, isOutput=False)
    mask8 = nc.declare_dram_parameter("mask8", [128, 8], BF16, isOutput=False)
    out2 = nc.declare_dram_parameter("out2", [2, 128, 64, 64], BF16, isOutput=True)

    with TileContext(nc) as tc:
        with (
            tc.tile_pool(name="const", bufs=1) as cpool,
            tc.tile_pool(name="persist", bufs=1) as ppool,
            tc.tile_pool(name="wstream", bufs=8) as wpool,
            tc.tile_pool(name="outstage", bufs=4) as opool,
        ):
            # ---- warm the ACT exp table before it is needed ----
            scr = cpool.tile([1, 2], F32)
            nc.gpsimd.memset(scr[:, 0:1], 0.0)
            nc.scalar.activation(scr[:, 1:2], scr[:, 0:1], ACT.Exp)

            # identities built on gpsimd (off the DMA queues)
            id_f32 = cpool.tile([16, 16], F32)
            make_identity(nc, id_f32)
            id_bf = cpool.tile([16, 16], BF16)
            make_identity(nc, id_bf)
            id_sb = cpool.tile([128, 128], BF16)
            make_identity(nc, id_sb)

            # sync HWDGE queue: attention params, then the weight stream
            sk_sb = cpool.tile([112, 2, 28], BF16)
            nc.sync.dma_start(out=sk_sb, in_=skb[:])
            w1_sb = [cpool.tile([112, 14, 196], BF16, name=f"w1_{i}") for i in (0, 1)]
            nc.sync.dma_start(out=w1_sb[0], in_=w1b[0])
            w2_sb = cpool.tile([98, 2, 16], BF16)
            nc.sync.dma_start(out=w2_sb, in_=w2b[:])
            selk_sb = cpool.tile([16, 128], F32)
            nc.sync.dma_start(out=selk_sb, in_=selk[:])
            mask_sb = cpool.tile([128, 8], BF16)
            nc.sync.dma_start(out=mask_sb, in_=mask8[:])
            bias_sb = cpool.tile([16, 128], F32)
            nc.sync.dma_start(out=bias_sb, in_=biash[:])

            # scalar HWDGE queue: other w1 half, then x (ci=0 halves first)
            nc.scalar.dma_start(out=w1_sb[1], in_=w1b[1])
            xpad = [[None, None], [None, None]]
            for ci in range(2):
                for b in range(2):
                    pad = ppool.tile([128, 66, 66], BF16, name=f"xpad{b}{ci}")
                    nc.scalar.dma_start(out=pad, in_=xp[b, ci])
                    xpad[b][ci] = pad

            bias_b = ppool.tile([128, 2], F32)   # aggregated bias per sample
            rhs_sel = ppool.tile([128, 16], BF16)  # att[b,k]*d(o8,o8')

            with tc.tile_pool(name="psA", bufs=2, space="PSUM") as psA:
                # ---- hidden = relu(pooled @ w1.T) for both samples ----
                ps_h = psA.tile([2, 196], F32, tag="att_ps")
                for c in range(28):
                    nc.tensor.matmul(
                        ps_h,
                        sk_sb[:, :, c],                  # [112, 2]
                        w1_sb[c // 14][:, c % 14, :],    # [112, 196]
                        start=(c == 0),
                        stop=(c == 27),
                    )
                hdn = ppool.tile([2, 196], BF16)
                nc.vector.tensor_relu(hdn, ps_h)

                # transpose hdn chunks: [2, 98] -> [98, 2]
                hdnT = ppool.tile([98, 2, 2], BF16)
                for c2 in range(2):
                    ps_t = psA.tile([98, 2], BF16, tag="att_ps", name="ps_t")
                    nc.tensor.transpose(
                        ps_t, hdn[:, 98 * c2 : 98 * (c2 + 1)], id_bf[:2, :2]
                    )
                    nc.vector.tensor_copy(hdnT[:, c2, :], ps_t)

                # logits = hdn @ w2.T   -> [2, 16]
                ps_l = psA.tile([2, 16], F32, tag="att_ps", name="ps_l")
                for c2 in range(2):
                    nc.tensor.matmul(
                        ps_l,
                        hdnT[:, c2, :],          # [98, 2]
                        w2_sb[:, c2, :],         # [98, 16]
                        start=(c2 == 0),
                        stop=(c2 == 1),
                    )

                # softmax(logits / T); T=30 keeps exp well in range, so no
                # max-subtraction is needed and exp reads PSUM directly.
                att_e = ppool.tile([2, 16], F32)
                nc.scalar.activation(att_e, ps_l, ACT.Exp, scale=1.0 / TEMPERATURE)
                sm = ppool.tile([2, 1], F32)
                nc.vector.tensor_reduce(sm, att_e, axis=AX.X, op=OP.add)
                rec = ppool.tile([2, 1], F32)
                nc.vector.reciprocal(rec, sm)
                att_sb = ppool.tile([2, 16], F32)
                nc.vector.tensor_scalar_mul(att_sb, att_e, rec)

                # attT[k, b] = att[b, k]
                ps_at = psA.tile([16, 2], F32, tag="att_ps", name="ps_at")
                nc.tensor.transpose(ps_at, att_sb, id_f32[:2, :2])
                attT = ppool.tile([16, 2], F32)
                nc.vector.tensor_copy(attT, ps_at)

                # aggregated bias: bias_b[:, b] = sum_k att[b,k] bias[k, :]
                ps_ab = psA.tile([128, 2], F32, tag="att_ps", name="ps_ab")
                nc.tensor.matmul(ps_ab, bias_sb, attT, start=True, stop=True)
                nc.vector.tensor_copy(bias_b, ps_ab)

                # attc[(o8,k), b] = att[b, k]  (replicated over o8)
                ps_ac = psA.tile([128, 2], F32, tag="att_ps", name="ps_ac")
                nc.tensor.matmul(ps_ac, selk_sb, attT, start=True, stop=True)
                attc = ppool.tile([128, 2], F32)
                nc.vector.tensor_copy(attc, ps_ac)

                # rhs_sel[(o8,k), (b,o8')] = att[b,k] * d(o8,o8')
                for b in range(2):
                    nc.vector.tensor_scalar_mul(
                        rhs_sel[:, 8 * b : 8 * b + 8], mask_sb, attc[:, b : b + 1]
                    )

            # ---------------- PE weight mix ----------------
            # acc[b][ci][il, t, o] = sum_k att[b,k] * W[k, o(half), 128ci+il, t]
            acc = [[None, None], [None, None]]
            for b in range(2):
                for ci in range(2):
                    acc[b][ci] = ppool.tile([128, 9, 128], BF16, name=f"acc{b}{ci}")

            with (
                tc.tile_pool(name="psM", bufs=2, space="PSUM") as psM,
                tc.tile_pool(name="psC", bufs=6, space="PSUM") as psC,
            ):
                wchunk = {}
                for ci in range(2):
                    for q in range(4):  # 4-slab DMA chunks, all queued upfront
                        wc = wpool.tile(
                            [128, 4, 9, 128], BF16, tag="wslab", name="wslab"
                        )
                        nc.sync.dma_start(
                            out=wc,
                            in_=wm[ci, 4 * q : 4 * q + 4].rearrange(
                                "ob p t il -> p ob t il"
                            ),
                        )
                        wchunk[(ci, q)] = wc
                for ci in range(2):
                    for g, (ob0, nob) in enumerate(OB_GROUPS):
                        pm = psM.tile([128, 512], F32, tag="mix", name="pm")
                        for ol in range(nob):
                            ob = ob0 + ol
                            wc = wchunk[(ci, ob // 4)]
                            for t in range(9):
                                col = ol * 144 + t * 16
                                nc.tensor.matmul(
                                    pm[:, col : col + 16],
                                    wc[:, ob % 4, t, :],
                                    rhs_sel,
                                    start=True,
                                    stop=True,
                                )
                        # drain the bank: [128, (ol, t, b, o8)] -> acc[b][ci]
                        view = pm[:, : nob * 144].rearrange(
                            "p (ol t e) -> p t ol e", ol=nob, t=9, e=16
                        )
                        for b in range(2):
                            nc.vector.tensor_copy(
                                acc[b][ci][:, :, 8 * ob0 : 8 * (ob0 + nob)],
                                view[:, :, :, 8 * b : 8 * b + 8],
                            )
                    if ci == 0:
                        # residual as conv identity tap on this core's own half
                        for b in range(2):
                            nc.vector.tensor_add(
                                acc[b][0][:, 4, :], acc[b][0][:, 4, :], id_sb
                            )

                # ---------------- conv + epilogue ----------------
                # pairs of adjacent pixel tiles share one [128, 1024] bf16
                # staging tile and one output DMA.
                pairs = [(b, 2 * pp) for b in range(2) for pp in range(4)]
                for w0 in range(0, 8, 3):           # waves of 3 pairs = 6 banks
                    wave = pairs[w0 : w0 + 3]
                    pcv = {}
                    for b, pt0 in wave:             # phase A: ci=0 taps
                        for pt in (pt0, pt0 + 1):
                            r0 = 8 * pt
                            p = psC.tile([128, 512], F32, tag="cv", name="pcv")
                            pcv[(b, pt)] = p
                            for t in range(9):
                                ty, tx = t // 3, t % 3
                                nc.tensor.matmul(
                                    p,
                                    acc[b][0][:, t, :],
                                    xpad[b][0][:, r0 + ty : r0 + ty + 8, tx : tx + 64],
                                    start=(t == 0),
                                    stop=False,
                                )
                    for b, pt0 in wave:             # phase B: ci=1 + epilogue
                        osb = opool.tile([128, 1024], BF16, tag="osb", name="osb")
                        for j, pt in enumerate((pt0, pt0 + 1)):
                            r0 = 8 * pt
                            p = pcv[(b, pt)]
                            for t in range(9):
                                ty, tx = t // 3, t % 3
                                nc.tensor.matmul(
                                    p,
                                    acc[b][1][:, t, :],
                                    xpad[b][1][:, r0 + ty : r0 + ty + 8, tx : tx + 64],
                                    start=False,
                                    stop=(t == 8),
                                )
                            nc.scalar.activation(
                                osb[:, 512 * j : 512 * (j + 1)], p,
                                ACT.Identity, bias=bias_b[:, b : b + 1],
                            )
                        nc.sync.dma_start(
                            out=out2[b, :, 8 * pt0 : 8 * pt0 + 16, :], in_=osb
                        )

    _split_multiwaits(nc)
    return nc


def _split_multiwaits(nc: bass.Bass):
    """This walrus build gives every TPB instruction exactly ONE sync-wait
    slot.  Tile emits multi-wait instructions; split the extras onto
    same-engine NoOp carriers inserted immediately before."""
    import bass_rust

    cnt = 0
    for fn in nc.m.functions:
        for blk in fn.blocks:
            out = []
            for ins in blk.instructions:
                si = getattr(ins, "sync_info", None)
                if si is not None and len(si.on_wait) > 1:
                    waits = list(si.on_wait)
                    for w in waits[:-1]:
                        cnt += 1
                        out.append(
                            bass_rust.InstNoOp(
                                name=f"waitcarrier-{cnt}",
                                engine=ins.engine,
                                ins=[],
                                outs=[],
                                sync_info=mybir.SyncInfo(
                                    on_wait=[w], on_update=[]
                                ),
                            )
                        )
                    ins.sync_info = mybir.SyncInfo(
                        on_wait=[waits[-1]], on_update=list(si.on_update)
                    )
                out.append(ins)
            blk.instructions = out


_PROGRAM = None


def _get_program():
    global _PROGRAM
    if _PROGRAM is None:
        _PROGRAM = build_program()
    return _PROGRAM


def _prepare_in_maps(x, scene_knowledge, weight, bias, att_w1, att_w2):
    x = np.ascontiguousarray(x, dtype=np.float32)
    scene_knowledge = np.ascontiguousarray(scene_knowledge, dtype=np.float32)
    weight = np.ascontiguousarray(weight, dtype=np.float32)
    bias = np.ascontiguousarray(bias, dtype=np.float32)
    att_w1 = np.ascontiguousarray(att_w1, dtype=np.float32)
    att_w2 = np.ascontiguousarray(att_w2, dtype=np.float32)

    K = 16
    # Wm[h, ci, ob, (o8,k), t, il] = weight[k, 128h+8ob+o8, 128ci+il, ty, tx]
    Wm = (
        weight.reshape(K, 2, 16, 8, 2, 128, 3, 3)  # k h ob o8 ci il ty tx
        .transpose(1, 4, 2, 3, 0, 6, 7, 5)         # h ci ob o8 k ty tx il
        .astype(BF)
        .reshape(2, 2, 16, 128, 9, 128)
    )

    # x zero-padded to 66x66 in bf16: [bs, ci, il, 66, 66]
    xpad = np.zeros((8, 2, 128, 66, 66), BF)
    xpad[:, :, :, 1:65, 1:65] = x.reshape(8, 2, 128, 64, 64)

    # fold 2x2 avg-pool into w1:  w1p[j, r*56+c] = 0.25 * w1[j, r//2, c//2]
    w1p = 0.25 * np.repeat(
        np.repeat(att_w1.reshape(196, 28, 28), 2, axis=1), 2, axis=2
    ).reshape(196, 3136)
    # w1b[half, p, c, j] = w1p[j, p*28 + half*14 + c]
    w1bf = np.ascontiguousarray(
        w1p.T.reshape(112, 2, 14, 196).transpose(1, 0, 2, 3).astype(BF)
    )
    # w2b[p, c2, e] = att_w2[e, c2*98 + p]
    w2bf = np.ascontiguousarray(
        att_w2.T.reshape(2, 98, 16).transpose(1, 0, 2).astype(BF)
    )
    # skb[p, b, c] = scene[b, p*28 + c]
    skbf = np.ascontiguousarray(
        scene_knowledge.reshape(8, 112, 28).transpose(1, 0, 2)
    ).astype(BF)

    p128 = np.arange(128)
    selkm = np.zeros((16, 128), np.float32)
    selkm[p128 % 16, p128] = 1.0
    mask8m = np.zeros((128, 8), np.float32)
    mask8m[p128, p128 // 16] = 1.0
    mask8m = mask8m.astype(BF)

    in_maps = []
    for c in range(NCORES):
        g, h = c // 2, c % 2
        perm = [h, 1 - h]  # ci chunk 0 == this core's output half
        in_maps.append(
            {
                "xp": np.ascontiguousarray(xpad[2 * g : 2 * g + 2][:, perm]),
                "skb": np.ascontiguousarray(skbf[:, 2 * g : 2 * g + 2]),
                "w1b": w1bf,
                "w2b": w2bf,
                "wm": np.ascontiguousarray(Wm[h][perm]),
                "biash": np.ascontiguousarray(bias[:, 128 * h : 128 * (h + 1)]),
                "selk": selkm,
                "mask8": mask8m,
            }
        )
    return in_maps


def _assemble(results):
    out = np.empty((8, 256, 64, 64), np.float32)
    for c in range(NCORES):
        g, h = c // 2, c % 2
        out[2 * g : 2 * g + 2, 128 * h : 128 * (h + 1)] = (
            results[c]["out2"].astype(np.float32)
        )
    return out


def run(inputs: dict, trace: bool = False, tmpdir: str | None = None):
    from concourse.bass_utils import run_bass_kernel_spmd

    nc = _get_program()
    in_maps = _prepare_in_maps(**inputs)
    res = run_bass_kernel_spmd(
        nc, in_maps, core_ids=list(range(NCORES)), trace=trace, tmpdir=tmpdir
    )
    return _assemble(res.results), res


def kernel(**inputs) -> np.ndarray:
    out, _ = run(inputs, trace=False)
    return out


# revision 9
# speedup vs baseline: 1.8470x; 1.1520x over previous
"""Trainium2 Bass kernel for the CondConv-style dense CNN.

Model (per sample b):
  att[b]  = softmax(MLP(avgpool(scene_knowledge[b])) / 30)        # [16]
  agg_w   = sum_k att[b,k] * weight[k]                            # [256,256,3,3]
  out[b]  = conv3x3_same(x[b], agg_w) + att[b] @ bias + x[b]

Sharding: 8 cores = 4 sample-pairs (g) x 2 output-channel halves (h).
Each core processes 2 samples and 128 output channels.

The expert mix runs on the PE as selector matmuls: stationary operand is a
weight chunk [(o8,k)=128, il=128] (o8 = output channel mod-8 block, k =
expert), moving operand is a [128, 16] selector rhs_sel[(o8,k),(b,o8')] =
att[b,k] * delta(o8,o8'), so each chunk matmul emits aggregated weights
for 8 output channels x both samples straight into PSUM with partition =
il — the exact lhsT orientation the conv needs.  DVE only drains PSUM to
bf16 SBUF.  The residual is folded into the conv as a +I center tap.
"""

import sys
import numpy as np

sys.path.insert(0, "/opt/trn_rl_repo")

import ml_dtypes

import concourse.bass as bass
import concourse.mybir as mybir
from concourse.tile import TileContext
from concourse.masks import make_identity

F32 = mybir.dt.float32
BF16 = mybir.dt.bfloat16
AX = mybir.AxisListType
OP = mybir.AluOpType
ACT = mybir.ActivationFunctionType

TEMPERATURE = 30.0
NCORES = 8
BF = ml_dtypes.bfloat16

# ob groups per PSUM bank during the mix: [3, 3, 3, 3, 3, 1]
OB_GROUPS = [(0, 3), (3, 3), (6, 3), (9, 3), (12, 3), (15, 1)]


def build_program() -> bass.Bass:
    nc = bass.Bass()

    xp = nc.declare_dram_parameter("xp", [2, 2, 128, 66, 66], BF16, isOutput=False)
    skb = nc.declare_dram_parameter("skb", [112, 2, 28], BF16, isOutput=False)
    w1b = nc.declare_dram_parameter("w1b", [2, 112, 14, 196], BF16, isOutput=False)
    w2b = nc.declare_dram_parameter("w2b", [98, 2, 16], BF16, isOutput=False)
    wm = nc.declare_dram_parameter("wm", [2, 16, 128, 9, 128], BF16, isOutput=False)
    biash = nc.declare_dram_parameter("biash", [16, 128], F32, isOutput=False)
    selk = nc.declare_dram_parameter("selk", [16, 128], F32, isOutput=False)
    mask8 = nc.declare_dram_parameter("mask8", [128, 8], BF16, isOutput=False)
    out2 = nc.declare_dram_parameter("out2", [2, 128, 64, 64], BF16, isOutput=True)

    with TileContext(nc) as tc:
        with (
            tc.tile_pool(name="const", bufs=1) as cpool,
            tc.tile_pool(name="persist", bufs=1) as ppool,
            tc.tile_pool(name="wstream", bufs=8) as wpool,
            tc.tile_pool(name="outstage", bufs=4) as opool,
        ):
            # ---- warm the ACT exp table before it is needed ----
            scr = cpool.tile([1, 2], F32)
            nc.gpsimd.memset(scr[:, 0:1], 0.0)
            nc.scalar.activation(scr[:, 1:2], scr[:, 0:1], ACT.Exp)

            # identities built on gpsimd (off the DMA queues)
            id_f32 = cpool.tile([16, 16], F32)
            make_identity(nc, id_f32)
            id_bf = cpool.tile([16, 16], BF16)
            make_identity(nc, id_bf)
            id_sb = cpool.tile([128, 128], BF16)
            make_identity(nc, id_sb)

            # sync HWDGE queue: attention params, then the weight stream
            sk_sb = cpool.tile([112, 2, 28], BF16)
            nc.sync.dma_start(out=sk_sb, in_=skb[:])
            w1_sb = [cpool.tile([112, 14, 196], BF16, name=f"w1_{i}") for i in (0, 1)]
            nc.sync.dma_start(out=w1_sb[0], in_=w1b[0])
            w2_sb = cpool.tile([98, 2, 16], BF16)
            nc.sync.dma_start(out=w2_sb, in_=w2b[:])
            selk_sb = cpool.tile([16, 128], F32)
            nc.sync.dma_start(out=selk_sb, in_=selk[:])
            mask_sb = cpool.tile([128, 8], BF16)
            nc.sync.dma_start(out=mask_sb, in_=mask8[:])
            bias_sb = cpool.tile([16, 128], F32)
            nc.sync.dma_start(out=bias_sb, in_=biash[:])

            # scalar HWDGE queue: other w1 half, then x (ci=0 halves first)
            nc.scalar.dma_start(out=w1_sb[1], in_=w1b[1])
            xpad = [[None, None], [None, None]]
            for ci in range(2):
                for b in range(2):
                    pad = ppool.tile([128, 66, 66], BF16, name=f"xpad{b}{ci}")
                    nc.scalar.dma_start(out=pad, in_=xp[b, ci])
                    xpad[b][ci] = pad

            bias_b = ppool.tile([128, 2], F32)   # aggregated bias per sample
            rhs_sel = ppool.tile([128, 16], BF16)  # att[b,k]*d(o8,o8')

            with tc.tile_pool(name="psA", bufs=2, space="PSUM") as psA:
                # ---- hidden = relu(pooled @ w1.T) for both samples ----
                ps_h = psA.tile([2, 196], F32, tag="att_ps")
                for c in range(28):
                    nc.tensor.matmul(
                        ps_h,
                        sk_sb[:, :, c],                  # [112, 2]
                        w1_sb[c // 14][:, c % 14, :],    # [112, 196]
                        start=(c == 0),
                        stop=(c == 27),
                    )
                hdn = ppool.tile([2, 196], BF16)
                nc.vector.tensor_relu(hdn, ps_h)

                # transpose hdn chunks: [2, 98] -> [98, 2]
                hdnT = ppool.tile([98, 2, 2], BF16)
                for c2 in range(2):
                    ps_t = psA.tile([98, 2], BF16, tag="att_ps", name="ps_t")
                    nc.tensor.transpose(
                        ps_t, hdn[:, 98 * c2 : 98 * (c2 + 1)], id_bf[:2, :2]
                    )
                    nc.vector.tensor_copy(hdnT[:, c2, :], ps_t)

                # logits = hdn @ w2.T   -> [2, 16]
                ps_l = psA.tile([2, 16], F32, tag="att_ps", name="ps_l")
                for c2 in range(2):
                    nc.tensor.matmul(
                        ps_l,
                        hdnT[:, c2, :],          # [98, 2]
                        w2_sb[:, c2, :],         # [98, 16]
                        start=(c2 == 0),
                        stop=(c2 == 1),
                    )

                # softmax(logits / T); T=30 keeps exp well in range, so no
                # max-subtraction is needed and exp reads PSUM directly.
                att_e = ppool.tile([2, 16], F32)
                nc.scalar.activation(att_e, ps_l, ACT.Exp, scale=1.0 / TEMPERATURE)
                sm = ppool.tile([2, 1], F32)
                nc.vector.tensor_reduce(sm, att_e, axis=AX.X, op=OP.add)
                rec = ppool.tile([2, 1], F32)
                nc.vector.reciprocal(rec, sm)
                att_sb = ppool.tile([2, 16], F32)
                nc.vector.tensor_scalar_mul(att_sb, att_e, rec)

                # attT[k, b] = att[b, k]
                ps_at = psA.tile([16, 2], F32, tag="att_ps", name="ps_at")
                nc.tensor.transpose(ps_at, att_sb, id_f32[:2, :2])
                attT = ppool.tile([16, 2], F32)
                nc.vector.tensor_copy(attT, ps_at)

                # aggregated bias: bias_b[:, b] = sum_k att[b,k] bias[k, :]
                ps_ab = psA.tile([128, 2], F32, tag="att_ps", name="ps_ab")
                nc.tensor.matmul(ps_ab, bias_sb, attT, start=True, stop=True)
                nc.vector.tensor_copy(bias_b, ps_ab)

                # attc[(o8,k), b] = att[b, k]  (replicated over o8)
                ps_ac = psA.tile([128, 2], F32, tag="att_ps", name="ps_ac")
                nc.tensor.matmul(ps_ac, selk_sb, attT, start=True, stop=True)
                attc = ppool.tile([128, 2], F32)
                nc.vector.tensor_copy(attc, ps_ac)

                # rhs_sel[(o8,k), (b,o8')] = att[b,k] * d(o8,o8')
                for b in range(2):
                    nc.vector.tensor_scalar_mul(
                        rhs_sel[:, 8 * b : 8 * b + 8], mask_sb, attc[:, b : b + 1]
                    )

            # ---------------- PE weight mix ----------------
            # acc[b][ci][il, t, o] = sum_k att[b,k] * W[k, o(half), 128ci+il, t]
            acc = [[None, None], [None, None]]
            for b in range(2):
                for ci in range(2):
                    acc[b][ci] = ppool.tile([128, 9, 128], BF16, name=f"acc{b}{ci}")

            with (
                tc.tile_pool(name="psM", bufs=3, space="PSUM") as psM,
                tc.tile_pool(name="psC", bufs=5, space="PSUM") as psC,
            ):
                wchunk = {}
                for ci in range(2):
                    for q in range(4):  # 4-slab DMA chunks, all queued upfront
                        wc = wpool.tile(
                            [128, 4, 9, 128], BF16, tag="wslab", name="wslab"
                        )
                        nc.sync.dma_start(
                            out=wc,
                            in_=wm[ci, 4 * q : 4 * q + 4].rearrange(
                                "ob p t il -> p ob t il"
                            ),
                        )
                        wchunk[(ci, q)] = wc
                for ci in range(2):
                    for g, (ob0, nob) in enumerate(OB_GROUPS):
                        pm = psM.tile([128, 512], F32, tag="mix", name="pm")
                        for ol in range(nob):
                            ob = ob0 + ol
                            wc = wchunk[(ci, ob // 4)]
                            for t in range(9):
                                col = ol * 144 + t * 16
                                nc.tensor.matmul(
                                    pm[:, col : col + 16],
                                    wc[:, ob % 4, t, :],
                                    rhs_sel,
                                    start=True,
                                    stop=True,
                                )
                        # drain the bank: [128, (ol, t, b, o8)] -> acc[b][ci]
                        view = pm[:, : nob * 144].rearrange(
                            "p (ol t e) -> p t ol e", ol=nob, t=9, e=16
                        )
                        for b in range(2):
                            nc.vector.tensor_copy(
                                acc[b][ci][:, :, 8 * ob0 : 8 * (ob0 + nob)],
                                view[:, :, :, 8 * b : 8 * b + 8],
                            )
                    if ci == 0:
                        # residual as conv identity tap on this core's own half
                        for b in range(2):
                            nc.vector.tensor_add(
                                acc[b][0][:, 4, :], acc[b][0][:, 4, :], id_sb
                            )

                # ---------------- conv + epilogue ----------------
                # pairs of adjacent pixel tiles share one [128, 1024] bf16
                # staging tile and one output DMA.
                pairs = [(b, 2 * pp) for b in range(2) for pp in range(4)]
                for w0 in range(0, 8, 2):           # waves of 2 pairs = 4 banks
                    wave = pairs[w0 : w0 + 2]
                    pcv = {}
                    for b, pt0 in wave:             # phase A: ci=0 taps
                        for pt in (pt0, pt0 + 1):
                            r0 = 8 * pt
                            p = psC.tile([128, 512], F32, tag="cv", name="pcv")
                            pcv[(b, pt)] = p
                            for t in range(9):
                                ty, tx = t // 3, t % 3
                                nc.tensor.matmul(
                                    p,
                                    acc[b][0][:, t, :],
                                    xpad[b][0][:, r0 + ty : r0 + ty + 8, tx : tx + 64],
                                    start=(t == 0),
                                    stop=False,
                                )
                    for b, pt0 in wave:             # phase B: ci=1 + epilogue
                        osb = opool.tile([128, 1024], BF16, tag="osb", name="osb")
                        for j, pt in enumerate((pt0, pt0 + 1)):
                            r0 = 8 * pt
                            p = pcv[(b, pt)]
                            for t in range(9):
                                ty, tx = t // 3, t % 3
                                nc.tensor.matmul(
                                    p,
                                    acc[b][1][:, t, :],
                                    xpad[b][1][:, r0 + ty : r0 + ty + 8, tx : tx + 64],
                                    start=False,
                                    stop=(t == 8),
                                )
                            nc.scalar.activation(
                                osb[:, 512 * j : 512 * (j + 1)], p,
                                ACT.Identity, bias=bias_b[:, b : b + 1],
                            )
                        nc.sync.dma_start(
                            out=out2[b, :, 8 * pt0 : 8 * pt0 + 16, :], in_=osb
                        )

    _split_multiwaits(nc)
    return nc


def _split_multiwaits(nc: bass.Bass):
    """This walrus build gives every TPB instruction exactly ONE sync-wait
    slot.  Tile emits multi-wait instructions; split the extras onto
    same-engine NoOp carriers inserted immediately before."""
    import bass_rust

    cnt = 0
    for fn in nc.m.functions:
        for blk in fn.blocks:
            out = []
            for ins in blk.instructions:
                si = getattr(ins, "sync_info", None)
                if si is not None and len(si.on_wait) > 1:
                    waits = list(si.on_wait)
                    for w in waits[:-1]:
                        cnt += 1
                        out.append(
                            bass_rust.InstNoOp(
                                name=f"waitcarrier-{cnt}",
                                engine=ins.engine,
                                ins=[],
                                outs=[],
                                sync_info=mybir.SyncInfo(
                                    on_wait=[w], on_update=[]
                                ),
                            )
                        )
                    ins.sync_info = mybir.SyncInfo(
                        on_wait=[waits[-1]], on_update=list(si.on_update)
                    )
                out.append(ins)
            blk.instructions = out


_PROGRAM = None


def _get_program():
    global _PROGRAM
    if _PROGRAM is None:
        _PROGRAM = build_program()
    return _PROGRAM


def _prepare_in_maps(x, scene_knowledge, weight, bias, att_w1, att_w2):
    x = np.ascontiguousarray(x, dtype=np.float32)
    scene_knowledge = np.ascontiguousarray(scene_knowledge, dtype=np.float32)
    weight = np.ascontiguousarray(weight, dtype=np.float32)
    bias = np.ascontiguousarray(bias, dtype=np.float32)
    att_w1 = np.ascontiguousarray(att_w1, dtype=np.float32)
    att_w2 = np.ascontiguousarray(att_w2, dtype=np.float32)

    K = 16
    # Wm[h, ci, ob, (o8,k), t, il] = weight[k, 128h+8ob+o8, 128ci+il, ty, tx]
    Wm = (
        weight.reshape(K, 2, 16, 8, 2, 128, 3, 3)  # k h ob o8 ci il ty tx
        .transpose(1, 4, 2, 3, 0, 6, 7, 5)         # h ci ob o8 k ty tx il
        .astype(BF)
        .reshape(2, 2, 16, 128, 9, 128)
    )

    # x zero-padded to 66x66 in bf16: [bs, ci, il, 66, 66]
    xpad = np.zeros((8, 2, 128, 66, 66), BF)
    xpad[:, :, :, 1:65, 1:65] = x.reshape(8, 2, 128, 64, 64)

    # fold 2x2 avg-pool into w1:  w1p[j, r*56+c] = 0.25 * w1[j, r//2, c//2]
    w1p = 0.25 * np.repeat(
        np.repeat(att_w1.reshape(196, 28, 28), 2, axis=1), 2, axis=2
    ).reshape(196, 3136)
    # w1b[half, p, c, j] = w1p[j, p*28 + half*14 + c]
    w1bf = np.ascontiguousarray(
        w1p.T.reshape(112, 2, 14, 196).transpose(1, 0, 2, 3).astype(BF)
    )
    # w2b[p, c2, e] = att_w2[e, c2*98 + p]
    w2bf = np.ascontiguousarray(
        att_w2.T.reshape(2, 98, 16).transpose(1, 0, 2).astype(BF)
    )
    # skb[p, b, c] = scene[b, p*28 + c]
    skbf = np.ascontiguousarray(
        scene_knowledge.reshape(8, 112, 28).transpose(1, 0, 2)
    ).astype(BF)

    p128 = np.arange(128)
    selkm = np.zeros((16, 128), np.float32)
    selkm[p128 % 16, p128] = 1.0
    mask8m = np.zeros((128, 8), np.float32)
    mask8m[p128, p128 // 16] = 1.0
    mask8m = mask8m.astype(BF)

    in_maps = []
    for c in range(NCORES):
        g, h = c // 2, c % 2
        perm = [h, 1 - h]  # ci chunk 0 == this core's output half
        in_maps.append(
            {
                "xp": np.ascontiguousarray(xpad[2 * g : 2 * g + 2][:, perm]),
                "skb": np.ascontiguousarray(skbf[:, 2 * g : 2 * g + 2]),
                "w1b": w1bf,
                "w2b": w2bf,
                "wm": np.ascontiguousarray(Wm[h][perm]),
                "biash": np.ascontiguousarray(bias[:, 128 * h : 128 * (h + 1)]),
                "selk": selkm,
                "mask8": mask8m,
            }
        )
    return in_maps


def _assemble(results):
    out = np.empty((8, 256, 64, 64), np.float32)
    for c in range(NCORES):
        g, h = c // 2, c % 2
        out[2 * g : 2 * g + 2, 128 * h : 128 * (h + 1)] = (
            results[c]["out2"].astype(np.float32)
        )
    return out


def run(inputs: dict, trace: bool = False, tmpdir: str | None = None):
    from concourse.bass_utils import run_bass_kernel_spmd

    nc = _get_program()
    in_maps = _prepare_in_maps(**inputs)
    res = run_bass_kernel_spmd(
        nc, in_maps, core_ids=list(range(NCORES)), trace=trace, tmpdir=tmpdir
    )
    return _assemble(res.results), res


def kernel(**inputs) -> np.ndarray:
    out, _ = run(inputs, trace=False)
    return out
